# revision 28
# baseline (speedup 1.0000x reference)
"""Trainium2 Bass kernel for nn_MetaNetLinearizedModel (collective-free).

Math (B=16, D=12288, F=768, HID=192, T=8):
    X = x.reshape(B, D)
    h1 = X @ W1 + b1                       [B, F]
    g  = gelu_tanh(h1); gp = gelu_tanh'(h1)
    feats = g @ W2 + b2                    [B, F]
    mh = relu(feats @ mW1.T + mb1)         [B, HID]
    coefs = mh @ mW2.T + mb2               [B, T]
    dh1_t = X @ dW1[t] + db1[t]
    dout  = sum_t coefs[:,t] * ((gp * dh1_t) @ W2 + g @ dW2[t] + db2[t])
    out   = feats + dout

Per-core plan (8 cores, NO collectives -> no cc barrier / dead window):
  - All big matmuls run fp8 DoubleRow with 2-term residual splits:
      x  = Xq + Xr/32        (Xq = fp8(x), Xr = fp8(32*(x-Xq)))
      W1 = (W1q + W1r/32)/64 (W1q = fp8(64*W1), W1r = fp8(2048*resid))
    Per k-pair, ONE stationary [128,2,32] = [Xq|Xr] feeds 6 DoubleRow
    matmuls: hA = [Xq|Xr]@W1q, hB = [Xq|Xr]@W1r, U = [Xq|Xr]@dW1q,
    each accumulating [32, 512]+[32, 256] PSUM.
  - Cross-partition folds (rows 16:32 scaled 1/32 into rows 0:16) and
    the b1/db1 bias adds happen via one [33,16] selection matmul per
    path (the 33rd row carries the bias).
    h1 ~ fold(hA) + hB[0:16]/2048  -> bf16-or-better accuracy for
    h1/g/gp/feats/coefs; delta dW1 stays 1-term fp8 (error ~9e-3 rel,
    budget 2e-2).
  - W2 / dW2 / meta-net / stationary casts bf16; g & gp via scalar
    engine activation LUTs (Gelu_apprx_tanh / Derivative_Gelu).
  - out_c = (c==0)*feats + coefs[:,c]*(z1 @ W2 + g @ dW2[c] + db2[c]),
    z1 = gp*(U + db1[c]); host sums the 8 core outputs.
"""
import sys

sys.path.insert(0, "/opt/trn_rl_repo")

import numpy as np
import ml_dtypes
import concourse.bass as bass
import concourse.bacc as bacc
import concourse.tile as tile
import concourse.mybir as mybir
from concourse import bass_utils


F32 = mybir.dt.float32
F32R = mybir.dt.float32r
BF16 = mybir.dt.bfloat16
FP8 = mybir.dt.float8e4
AF = mybir.ActivationFunctionType
OP = mybir.AluOpType
DR = mybir.MatmulPerfMode.DoubleRow

B = 16
D = 3 * 64 * 64        # 12288
F = 768
HID = 192
T = 8
NCORES = 8
KD = D // 128          # 96 k-tiles over D
KJ = KD // 2           # 48 pair-tiles
KF = F // 128          # 6
HF = F // 2            # 384 column half (F-split streaming)
KH = KF // 2           # 3
JJ = KJ // 2           # 24 two-pair chunks
WS = 64.0              # fp8 weight pre-scale
RS = 32.0              # residual pre-scale (x and W1)
GELU_C0 = float(np.sqrt(2.0 / np.pi))
GELU_A = 0.044715

# bc pack column offsets (fp32 [32, BCW])
BC_B1 = 0                   # width F
BC_B2 = BC_B1 + F           # width F
BC_DB2 = BC_B2 + F          # width F;  db2[c]
BC_MB1 = BC_DB2 + F         # width HID
BC_MB2 = BC_MB1 + HID       # width T
BC_SEL = BC_MB2 + T         # width T
BC_FB = BC_SEL + T          # width 1; 1.0 on core 0 else 0.0
BC_EYE = BC_FB + 1          # width 32
BCW = BC_EYE + 32

_CACHE = {}


def build():
    nc = bacc.Bacc("TRN2", target_bir_lowering=False, debug=False,
                   enable_asserts=False, num_devices=NCORES)

    XQR = nc.dram_tensor("xqr", [128, KD, 32], FP8, kind="ExternalInput")
    XTB = nc.dram_tensor("xtb", [128, KD, B], BF16, kind="ExternalInput")
    W1L = nc.dram_tensor("w1l", [JJ * 128, 4, HF], BF16, kind="ExternalInput")
    W1R = nc.dram_tensor("w1r", [JJ * 128, 4, HF], BF16, kind="ExternalInput")
    DW1L = nc.dram_tensor("dw1l", [JJ * 128, 4, HF], FP8, kind="ExternalInput")
    DW1R = nc.dram_tensor("dw1r", [JJ * 128, 4, HF], FP8, kind="ExternalInput")
    W2B = nc.dram_tensor("w2b", [F, F], BF16, kind="ExternalInput")
    DW2B = nc.dram_tensor("dw2b", [F, F], BF16, kind="ExternalInput")
    MW1T = nc.dram_tensor("mw1t", [128, KF * HID], BF16, kind="ExternalInput")
    MW2T = nc.dram_tensor("mw2t", [128, 2 * T], BF16, kind="ExternalInput")
    FSEL = nc.dram_tensor("fsel", [33, B], F32R, kind="ExternalInput")
    DB1R = nc.dram_tensor("db1r", [1, F], F32R, kind="ExternalInput")
    BC = nc.dram_tensor("bc", [32, BCW], F32, kind="ExternalInput")
    OT = nc.dram_tensor("ot", [B, F], F32, kind="ExternalOutput")

    with tile.TileContext(nc, num_cores=NCORES) as tc:
        with (
            tc.tile_pool(name="cst", bufs=1) as cst,
            tc.tile_pool(name="wrk", bufs=1) as wrk,
            tc.tile_pool(name="w1br", bufs=24) as w1br,
            tc.tile_pool(name="dw1p", bufs=28) as dw1p,
            tc.tile_pool(name="psu", bufs=1, space="PSUM") as psu,
            tc.tile_pool(name="pss", bufs=2, space="PSUM") as pss,
        ):
            # ---- priority loads (scalar queue) ----
            xqr_sb = cst.tile([128, KD, 32], FP8)
            nc.scalar.dma_start(xqr_sb[:], XQR.ap())
            xtb_sb = cst.tile([128, KD, B], BF16)
            nc.scalar.dma_start(xtb_sb[:], XTB.ap())
            bc_sb = cst.tile([32, BCW], F32)
            nc.scalar.dma_start(bc_sb[:], BC.ap())
            fsel_sb = cst.tile([33, B], F32R)
            nc.scalar.dma_start(fsel_sb[:], FSEL.ap())
            cmbu = wrk.tile([33, F], F32R)
            nc.scalar.dma_start(cmbu[32:33, :], DB1R.ap())
            # preload gelu table while the scalar engine is idle
            scr = wrk.tile([1, 8], F32)
            nc.scalar.activation(scr[0:1, 0:1], bc_sb[0:1, 0:1],
                                 AF.Gelu_apprx_tanh)
            # tail loads (consumed only after ~half the stream)
            w2b_sb = cst.tile([128, KF * F], BF16)
            for k in range(KF):
                nc.scalar.dma_start(w2b_sb[:, k * F:(k + 1) * F],
                                    W2B.ap()[k * 128:(k + 1) * 128, :])
            mw1t_sb = cst.tile([128, KF * HID], BF16)
            nc.scalar.dma_start(mw1t_sb[:], MW1T.ap())
            mw2t_sb = cst.tile([128, 2 * T], BF16)
            nc.scalar.dma_start(mw2t_sb[:], MW2T.ap())
            dw2b_sb = cst.tile([128, KF * F], BF16)
            for k in range(KF):
                nc.scalar.dma_start(dw2b_sb[:, k * F:(k + 1) * F],
                                    DW2B.ap()[k * 128:(k + 1) * 128, :])

            def bcs(col, w):
                return bc_sb[0:B, col:col + w]
            eye16 = bc_sb[0:B, BC_EYE:BC_EYE + B]

            # ---- main stream (F-split): bf16 h-chains + fp8 DoubleRow U.
            # Left half-columns stream first; the left tail (h1/gelu/
            # transposes/feats/z1) overlaps the right half's stream. ----
            hP = [psu.tile([B, HF], F32, name="hL"),
                  psu.tile([B, HF], F32, name="hR")]
            uP = [psu.tile([32, HF], F32, name="uL"),
                  psu.tile([32, HF], F32, name="uR")]
            h1c = wrk.tile([B, F], F32)
            g_c = wrk.tile([B, F], F32)
            gp_bt = wrk.tile([B, F], F32)
            gT_b = wrk.tile([128, KF * B], BF16)
            z1c = wrk.tile([B, F], F32)
            z1T = wrk.tile([128, KF * B], BF16)
            fa = psu.tile([B, HF], F32, name="fa")
            fb = psu.tile([B, HF], F32, name="fb")

            for ph, (W1X, DW1X) in enumerate(((W1L, DW1L), (W1R, DW1R))):
                co = ph * HF
                for j in range(JJ):
                    wt = w1br.tile([128, 4, HF], BF16, name="w1t", tag="w1t")
                    dt_ = dw1p.tile([128, 4, HF], FP8, name="dw1t", tag="dw1t")
                    if j % 2 == 0:
                        nc.sync.dma_start(wt[:],
                                          W1X.ap()[j * 128:(j + 1) * 128])
                        nc.gpsimd.dma_start(dt_[:],
                                            DW1X.ap()[j * 128:(j + 1) * 128])
                    else:
                        nc.gpsimd.dma_start(wt[:],
                                            W1X.ap()[j * 128:(j + 1) * 128])
                        nc.sync.dma_start(dt_[:],
                                          DW1X.ap()[j * 128:(j + 1) * 128])
                    st = (j == 0), (j == JJ - 1)
                    for h in range(4):
                        sb = xtb_sb[:, 4 * j + h, :]
                        nc.tensor.matmul(hP[ph][:], sb, wt[:, h, :],
                                         start=(st[0] and h == 0),
                                         stop=(st[1] and h == 3),
                                         skip_group_check=True)
                    for pp in range(2):
                        sq = xqr_sb[:, 4 * j + 2 * pp:4 * j + 2 * pp + 2, :]
                        nc.tensor.matmul(uP[ph][:], sq,
                                         dt_[:, 2 * pp:2 * pp + 2, :],
                                         perf_mode=DR,
                                         start=(st[0] and pp == 0),
                                         stop=(st[1] and pp == 1),
                                         skip_group_check=True)


                # per-half tail: h1, g, gp, g-transposes, partial feats,
                # U-fold, z1, z1-transposes. For ph=0 this overlaps the
                # right half's stream.
                nc.vector.tensor_add(h1c[:, co:co + HF], hP[ph][:],
                                     bc_sb[0:B, BC_B1 + co:BC_B1 + co + HF])
                nc.scalar.activation(g_c[:, co:co + HF], h1c[:, co:co + HF],
                                     AF.Gelu_apprx_tanh)
                nc.scalar.activation(gp_bt[:, co:co + HF],
                                     h1c[:, co:co + HF], AF.Derivative_Gelu)
                for fo in range(KH * ph, KH * (ph + 1)):
                    tp = pss.tile([128, B], F32, name="ps", tag="ps")
                    nc.tensor.transpose(tp[:],
                                        g_c[:, fo * 128:(fo + 1) * 128],
                                        eye16)
                    nc.vector.tensor_copy(gT_b[:, fo * B:(fo + 1) * B],
                                          tp[:])
                for k in range(KH * ph, KH * (ph + 1)):
                    nc.tensor.matmul(fa[:], gT_b[:, k * B:(k + 1) * B],
                                     w2b_sb[:, k * F:k * F + HF],
                                     start=(k == 0), stop=(k == KF - 1),
                                     skip_group_check=True)
                    nc.tensor.matmul(fb[:], gT_b[:, k * B:(k + 1) * B],
                                     w2b_sb[:, k * F + HF:(k + 1) * F],
                                     start=(k == 0), stop=(k == KF - 1),
                                     skip_group_check=True)
                nc.vector.tensor_scalar(cmbu[0:32, co:co + HF], uP[ph][:],
                                        1.0 / WS, None, OP.mult)
                uf = pss.tile([B, HF], F32, name="ps", tag="ps")
                nc.tensor.matmul(uf[:], fsel_sb[:], cmbu[:, co:co + HF],
                                 start=True, stop=True,
                                 skip_group_check=True)
                nc.vector.tensor_mul(z1c[:, co:co + HF], uf[:],
                                     gp_bt[:, co:co + HF])
                for fo in range(KH * ph, KH * (ph + 1)):
                    tp = pss.tile([128, B], F32, name="ps", tag="ps")
                    nc.tensor.transpose(tp[:],
                                        z1c[:, fo * 128:(fo + 1) * 128],
                                        eye16)
                    nc.vector.tensor_copy(z1T[:, fo * B:(fo + 1) * B],
                                          tp[:])

            feats = wrk.tile([B, F], F32)
            nc.vector.tensor_add(feats[:, 0:HF], fa[:], bcs(BC_B2, HF))
            nc.vector.tensor_add(feats[:, HF:F], fb[:],
                                 bc_sb[0:B, BC_B2 + HF:BC_B2 + F])
            # ---- feats = g @ W2 + b2 ; V = g @ dW2 (independent) ----
            fmask = wrk.tile([B, F], F32)
            nc.vector.tensor_scalar(fmask[:], feats[:],
                                    bc_sb[0:B, BC_FB:BC_FB + 1],
                                    None, OP.mult)

            featsT = wrk.tile([128, KF * B], BF16)
            for fo in range(KF):
                tp = pss.tile([128, B], F32, name="ps", tag="ps")
                nc.tensor.transpose(tp[:], feats[:, fo * 128:(fo + 1) * 128],
                                    eye16)
                nc.vector.tensor_copy(featsT[:, fo * B:(fo + 1) * B], tp[:])

            # ---- meta-net -> coefs -> csel ----
            mps = pss.tile([B, HID], F32, name="ps", tag="ps")
            for k in range(KF):
                nc.tensor.matmul(mps[:], featsT[:, k * B:(k + 1) * B],
                                 mw1t_sb[:, k * HID:(k + 1) * HID],
                                 start=(k == 0), stop=(k == KF - 1),
                                 skip_group_check=True)
            mh_bt = wrk.tile([B, HID], F32)
            mtmp = wrk.tile([B, HID], F32)
            nc.vector.tensor_add(mtmp[:], mps[:], bcs(BC_MB1, HID))
            nc.vector.tensor_relu(mh_bt[:], mtmp[:])

            mh_tr = wrk.tile([128, 2 * B], BF16)
            tp = pss.tile([128, B], F32, name="ps", tag="ps")
            nc.tensor.transpose(tp[:], mh_bt[:, 0:128], eye16)
            nc.vector.tensor_copy(mh_tr[:, 0:B], tp[:])
            tp = pss.tile([128, B], F32, name="ps", tag="ps")
            nc.tensor.transpose(tp[0:HID - 128, :], mh_bt[:, 128:HID], eye16)
            nc.vector.tensor_copy(mh_tr[0:HID - 128, B:2 * B],
                                  tp[0:HID - 128, :])

            cps = pss.tile([B, T], F32, name="ps", tag="ps")
            nc.tensor.matmul(cps[:], mh_tr[:, 0:B], mw2t_sb[:, 0:T],
                             start=True, stop=False, skip_group_check=True)
            nc.tensor.matmul(cps[:], mh_tr[0:HID - 128, B:2 * B],
                             mw2t_sb[0:HID - 128, T:2 * T],
                             start=False, stop=True, skip_group_check=True)
            coefs_bt = wrk.tile([B, T], F32)
            nc.vector.tensor_add(coefs_bt[:], cps[:], bcs(BC_MB2, T))
            csel = wrk.tile([B, 1], F32)
            cjunk = wrk.tile([B, T], F32)
            nc.vector.tensor_mul(cjunk[:], coefs_bt[:], bcs(BC_SEL, T))
            nc.vector.reduce_sum(csel[:], cjunk[:], axis=mybir.AxisListType.X)

            # ---- V = g @ dW2 + db2 ----
            v5 = pss.tile([B, 512], F32, name="ps", tag="ps")
            v2 = pss.tile([B, 256], F32, name="ps", tag="ps")
            for k in range(KF):
                nc.tensor.matmul(v5[:], gT_b[:, k * B:(k + 1) * B],
                                 dw2b_sb[:, k * F:k * F + 512],
                                 start=(k == 0), stop=(k == KF - 1),
                                 skip_group_check=True)
                nc.tensor.matmul(v2[:], gT_b[:, k * B:(k + 1) * B],
                                 dw2b_sb[:, k * F + 512:(k + 1) * F],
                                 start=(k == 0), stop=(k == KF - 1),
                                 skip_group_check=True)
            v_bt = wrk.tile([B, F], F32)
            nc.vector.tensor_add(v_bt[:, 0:512], v5[:], bcs(BC_DB2, 512))
            nc.vector.tensor_add(v_bt[:, 512:F], v2[:],
                                 bc_sb[0:B, BC_DB2 + 512:BC_DB2 + F])

            # ---- dout1 = z1 @ W2 ; ot = fb*feats + csel*(dout1 + V) ----
            o5 = pss.tile([B, 512], F32, name="ps", tag="ps")
            o2 = pss.tile([B, 256], F32, name="ps", tag="ps")
            for k in range(KF):
                nc.tensor.matmul(o5[:], z1T[:, k * B:(k + 1) * B],
                                 w2b_sb[:, k * F:k * F + 512],
                                 start=(k == 0), stop=(k == KF - 1),
                                 skip_group_check=True)
                nc.tensor.matmul(o2[:], z1T[:, k * B:(k + 1) * B],
                                 w2b_sb[:, k * F + 512:(k + 1) * F],
                                 start=(k == 0), stop=(k == KF - 1),
                                 skip_group_check=True)
            od = wrk.tile([B, F], F32)
            nc.vector.tensor_add(od[:, 0:512], o5[:], v_bt[:, 0:512])
            nc.vector.tensor_add(od[:, 512:F], o2[:], v_bt[:, 512:F])
            od2 = wrk.tile([B, F], F32)
            nc.vector.tensor_scalar(od2[:], od[:], csel[:], None, OP.mult)
            out2 = wrk.tile([B, F], F32)
            nc.vector.tensor_add(out2[:], od2[:], fmask[:])
            nc.scalar.dma_start(OT.ap(), out2[:])

    nc.compile()
    return nc


def _get_nc():
    if "nc" not in _CACHE:
        _CACHE["nc"] = build()
    return _CACHE["nc"]


def _prep_in_maps(x, W1, b1, W2, b2, mW1, mb1, mW2, mb2, dW1, db1, dW2, db2):
    f32 = np.float32
    bf16 = ml_dtypes.bfloat16
    fp8 = ml_dtypes.float8_e4m3
    X = np.ascontiguousarray(np.asarray(x, f32).reshape(B, D))
    XT = np.ascontiguousarray(X.T)                       # [D, B]
    xt3 = np.ascontiguousarray(
        XT.reshape(KD, 128, B).transpose(1, 0, 2))       # [128, KD, B]
    xq = xt3.astype(fp8)
    xr = ((xt3 - xq.astype(f32)) * RS).astype(fp8)
    xqr = np.empty((128, KD, 32), fp8)
    xqr[:, :, 0:B] = xq
    xqr[:, :, B:32] = xr
    xtb = xt3.astype(bf16)
    W1 = np.asarray(W1, f32)
    W2 = np.asarray(W2, f32)
    b1 = np.asarray(b1, f32); b2 = np.asarray(b2, f32)
    mb1 = np.asarray(mb1, f32); mb2 = np.asarray(mb2, f32)
    dW1 = np.asarray(dW1, f32); db1 = np.asarray(db1, f32)
    dW2 = np.asarray(dW2, f32); db2 = np.asarray(db2, f32)

    def pairs(w):  # [D, F] -> [KJ*128, 2, F]
        return np.ascontiguousarray(
            w.reshape(KJ, 2, 128, F).transpose(0, 2, 1, 3)
            .reshape(KJ * 128, 2, F))

    def pairs4(w):  # [D, F] -> [JJ*128, 4, F]
        return np.ascontiguousarray(
            w.reshape(JJ, 4, 128, F).transpose(0, 2, 1, 3)
            .reshape(JJ * 128, 4, F))

    w1p = pairs4(W1).astype(bf16)
    w1l = np.ascontiguousarray(w1p[:, :, 0:HF])
    w1rr = np.ascontiguousarray(w1p[:, :, HF:F])
    w2b = np.ascontiguousarray(W2).astype(bf16)
    mw1t = np.ascontiguousarray(
        np.asarray(mW1, f32).T.reshape(KF, 128, HID).transpose(1, 0, 2)
        .reshape(128, KF * HID)).astype(bf16)
    mw2tf = np.asarray(mW2, f32).T                       # [HID, T]
    mw2t = np.zeros((128, 2 * T), f32)
    mw2t[:, 0:T] = mw2tf[0:128, :]
    mw2t[0:HID - 128, T:2 * T] = mw2tf[128:HID, :]
    mw2t = mw2t.astype(bf16)
    fsel = np.zeros((33, B), f32)
    for b in range(B):
        fsel[b, b] = 1.0
        fsel[B + b, b] = 1.0 / RS
    fsel[32, :] = 1.0

    in_maps = []
    for c in range(NCORES):
        dwp = (pairs4(dW1[c]) * WS).astype(fp8)
        bc = np.zeros((32, BCW), f32)
        bc[0:B, BC_B1:BC_B1 + F] = b1[None, :]
        bc[0:B, BC_B2:BC_B2 + F] = b2[None, :]
        bc[0:B, BC_DB2:BC_DB2 + F] = db2[c][None, :]
        bc[0:B, BC_MB1:BC_MB1 + HID] = mb1[None, :]
        bc[0:B, BC_MB2:BC_MB2 + T] = mb2[None, :]
        bc[0:B, BC_SEL + c] = 1.0
        if c == 0:
            bc[0:B, BC_FB] = 1.0
        bc[0:32, BC_EYE:BC_EYE + 32] = np.eye(32, dtype=f32)
        in_maps.append({
            "xqr": xqr,
            "xtb": xtb,
            "w1l": w1l,
            "w1r": w1rr,
            "dw1l": np.ascontiguousarray(dwp[:, :, 0:HF]),
            "dw1r": np.ascontiguousarray(dwp[:, :, HF:F]),
            "w2b": w2b,
            "dw2b": np.ascontiguousarray(dW2[c]).astype(bf16),
            "mw1t": mw1t,
            "mw2t": mw2t,
            "fsel": fsel,
            "db1r": db1[c][None, :],
            "bc": bc,
        })
    return in_maps


def run(inputs, trace=False, trace_cores=None, tmpdir=None):
    nc = _get_nc()
    in_maps = _prep_in_maps(**inputs)
    res = bass_utils.run_bass_kernel_spmd(
        nc, in_maps, core_ids=list(range(NCORES)), trace=trace,
        trace_cores=trace_cores, tmpdir=tmpdir)
    acc = res.results[0]["ot"].astype(np.float64)
    for c in range(1, NCORES):
        acc = acc + res.results[c]["ot"].astype(np.float64)
    return acc.astype(np.float32), res


def kernel(**inputs):
    out, _ = run(inputs, trace=False)
    return out


# revision 29
# speedup vs baseline: 1.0418x; 1.0418x over previous
"""Trainium2 Bass kernel for nn_MetaNetLinearizedModel (collective-free).

Math (B=16, D=12288, F=768, HID=192, T=8):
    X = x.reshape(B, D)
    h1 = X @ W1 + b1                       [B, F]
    g  = gelu_tanh(h1); gp = gelu_tanh'(h1)
    feats = g @ W2 + b2                    [B, F]
    mh = relu(feats @ mW1.T + mb1)         [B, HID]
    coefs = mh @ mW2.T + mb2               [B, T]
    dh1_t = X @ dW1[t] + db1[t]
    dout  = sum_t coefs[:,t] * ((gp * dh1_t) @ W2 + g @ dW2[t] + db2[t])
    out   = feats + dout

Per-core plan (8 cores, NO collectives -> no cc barrier / dead window):
  - All big matmuls run fp8 DoubleRow with 2-term residual splits:
      x  = Xq + Xr/32        (Xq = fp8(x), Xr = fp8(32*(x-Xq)))
      W1 = (W1q + W1r/32)/64 (W1q = fp8(64*W1), W1r = fp8(2048*resid))
    Per k-pair, ONE stationary [128,2,32] = [Xq|Xr] feeds 6 DoubleRow
    matmuls: hA = [Xq|Xr]@W1q, hB = [Xq|Xr]@W1r, U = [Xq|Xr]@dW1q,
    each accumulating [32, 512]+[32, 256] PSUM.
  - Cross-partition folds (rows 16:32 scaled 1/32 into rows 0:16) and
    the b1/db1 bias adds happen via one [33,16] selection matmul per
    path (the 33rd row carries the bias).
    h1 ~ fold(hA) + hB[0:16]/2048  -> bf16-or-better accuracy for
    h1/g/gp/feats/coefs; delta dW1 stays 1-term fp8 (error ~9e-3 rel,
    budget 2e-2).
  - W2 / dW2 / meta-net / stationary casts bf16; g & gp via scalar
    engine activation LUTs (Gelu_apprx_tanh / Derivative_Gelu).
  - out_c = (c==0)*feats + coefs[:,c]*(z1 @ W2 + g @ dW2[c] + db2[c]),
    z1 = gp*(U + db1[c]); host sums the 8 core outputs.
"""
import sys

sys.path.insert(0, "/opt/trn_rl_repo")

import numpy as np
import ml_dtypes
import concourse.bass as bass
import concourse.bacc as bacc
import concourse.tile as tile
import concourse.mybir as mybir
from concourse import bass_utils


F32 = mybir.dt.float32
F32R = mybir.dt.float32r
BF16 = mybir.dt.bfloat16
FP8 = mybir.dt.float8e4
AF = mybir.ActivationFunctionType
OP = mybir.AluOpType
DR = mybir.MatmulPerfMode.DoubleRow

B = 16
D = 3 * 64 * 64        # 12288
F = 768
HID = 192
T = 8
NCORES = 8
KD = D // 128          # 96 k-tiles over D
KJ = KD // 2           # 48 pair-tiles
KF = F // 128          # 6
HF = F // 2            # 384 column half (F-split streaming)
KH = KF // 2           # 3
JJ = KJ // 2           # 24 two-pair chunks
WS = 64.0              # fp8 weight pre-scale
RS = 32.0              # residual pre-scale (x and W1)
GELU_C0 = float(np.sqrt(2.0 / np.pi))
GELU_A = 0.044715

# bc pack column offsets (fp32 [32, BCW])
BC_B1 = 0                   # width F
BC_B2 = BC_B1 + F           # width F
BC_DB2 = BC_B2 + F          # width F;  db2[c]
BC_MB1 = BC_DB2 + F         # width HID
BC_MB2 = BC_MB1 + HID       # width T
BC_SEL = BC_MB2 + T         # width T
BC_FB = BC_SEL + T          # width 1; 1.0 on core 0 else 0.0
BC_EYE = BC_FB + 1          # width 32
BCW = BC_EYE + 32

_CACHE = {}


def build():
    nc = bacc.Bacc("TRN2", target_bir_lowering=False, debug=False,
                   enable_asserts=False, num_devices=NCORES)

    XQR = nc.dram_tensor("xqr", [128, KD, 32], FP8, kind="ExternalInput")
    XTB = nc.dram_tensor("xtb", [128, KD, B], BF16, kind="ExternalInput")
    W1L = nc.dram_tensor("w1l", [JJ * 128, 4, HF], BF16, kind="ExternalInput")
    W1R = nc.dram_tensor("w1r", [JJ * 128, 4, HF], BF16, kind="ExternalInput")
    DW1L = nc.dram_tensor("dw1l", [JJ * 128, 4, HF], FP8, kind="ExternalInput")
    DW1R = nc.dram_tensor("dw1r", [JJ * 128, 4, HF], FP8, kind="ExternalInput")
    W2B = nc.dram_tensor("w2b", [F, F], BF16, kind="ExternalInput")
    DW2B = nc.dram_tensor("dw2b", [F, F], BF16, kind="ExternalInput")
    MW1T = nc.dram_tensor("mw1t", [128, KF * HID], BF16, kind="ExternalInput")
    MW2T = nc.dram_tensor("mw2t", [128, 2 * T], BF16, kind="ExternalInput")
    FSEL = nc.dram_tensor("fsel", [33, B], F32R, kind="ExternalInput")
    DB1R = nc.dram_tensor("db1r", [1, F], F32R, kind="ExternalInput")
    BC = nc.dram_tensor("bc", [32, BCW], F32, kind="ExternalInput")
    OT = nc.dram_tensor("ot", [B, F], F32, kind="ExternalOutput")

    with tile.TileContext(nc, num_cores=NCORES) as tc:
        with (
            tc.tile_pool(name="cst", bufs=1) as cst,
            tc.tile_pool(name="wrk", bufs=1) as wrk,
            tc.tile_pool(name="w1br", bufs=16) as w1br,
            tc.tile_pool(name="dw1p", bufs=20) as dw1p,
            tc.tile_pool(name="psu", bufs=1, space="PSUM") as psu,
            tc.tile_pool(name="pss", bufs=2, space="PSUM") as pss,
        ):
            # ---- priority loads (scalar queue) ----
            xqr_sb = cst.tile([128, KD, 32], FP8)
            nc.scalar.dma_start(xqr_sb[:], XQR.ap())
            xtb_sb = cst.tile([128, KD, B], BF16)
            nc.scalar.dma_start(xtb_sb[:], XTB.ap())
            bc_sb = cst.tile([32, BCW], F32)
            nc.scalar.dma_start(bc_sb[:], BC.ap())
            fsel_sb = cst.tile([33, B], F32R)
            nc.scalar.dma_start(fsel_sb[:], FSEL.ap())
            cmbu = wrk.tile([33, F], F32R)
            nc.scalar.dma_start(cmbu[32:33, :], DB1R.ap())
            # preload gelu table while the scalar engine is idle
            scr = wrk.tile([1, 8], F32)
            nc.scalar.activation(scr[0:1, 0:1], bc_sb[0:1, 0:1],
                                 AF.Gelu_apprx_tanh)
            # tail loads (consumed only after ~half the stream)
            w2b_sb = cst.tile([128, KF * F], BF16)
            for k in range(KF):
                nc.scalar.dma_start(w2b_sb[:, k * F:(k + 1) * F],
                                    W2B.ap()[k * 128:(k + 1) * 128, :])
            mw1t_sb = cst.tile([128, KF * HID], BF16)
            nc.scalar.dma_start(mw1t_sb[:], MW1T.ap())
            mw2t_sb = cst.tile([128, 2 * T], BF16)
            nc.scalar.dma_start(mw2t_sb[:], MW2T.ap())
            dw2b_sb = cst.tile([128, KF * F], BF16)
            for k in range(KF):
                nc.scalar.dma_start(dw2b_sb[:, k * F:(k + 1) * F],
                                    DW2B.ap()[k * 128:(k + 1) * 128, :])

            def bcs(col, w):
                return bc_sb[0:B, col:col + w]
            eye16 = bc_sb[0:B, BC_EYE:BC_EYE + B]

            # ---- main stream (F-split): bf16 h-chains + fp8 DoubleRow U.
            # Left half-columns stream first; the left tail (h1/gelu/
            # transposes/feats/z1) overlaps the right half's stream. ----
            hP = [psu.tile([B, HF], F32, name="hL"),
                  psu.tile([B, HF], F32, name="hR")]
            uP = [psu.tile([32, HF], F32, name="uL"),
                  psu.tile([32, HF], F32, name="uR")]
            h1c = wrk.tile([B, F], F32)
            g_c = wrk.tile([B, F], F32)
            gp_bt = wrk.tile([B, F], F32)
            gT_b = wrk.tile([128, KF * B], BF16)
            z1c = wrk.tile([B, F], F32)
            z1T = wrk.tile([128, KF * B], BF16)
            fa = psu.tile([B, HF], F32, name="fa")
            fb = psu.tile([B, HF], F32, name="fb")

            for ph, (W1X, DW1X) in enumerate(((W1L, DW1L), (W1R, DW1R))):
                co = ph * HF
                for j in range(JJ):
                    wt = w1br.tile([128, 4, HF], BF16, name="w1t", tag="w1t")
                    dt_ = dw1p.tile([128, 4, HF], FP8, name="dw1t", tag="dw1t")
                    if j % 2 == 0:
                        nc.sync.dma_start(wt[:],
                                          W1X.ap()[j * 128:(j + 1) * 128])
                        nc.gpsimd.dma_start(dt_[:],
                                            DW1X.ap()[j * 128:(j + 1) * 128])
                    else:
                        nc.gpsimd.dma_start(wt[:],
                                            W1X.ap()[j * 128:(j + 1) * 128])
                        nc.sync.dma_start(dt_[:],
                                          DW1X.ap()[j * 128:(j + 1) * 128])
                    st = (j == 0), (j == JJ - 1)
                    for h in range(4):
                        sb = xtb_sb[:, 4 * j + h, :]
                        nc.tensor.matmul(hP[ph][:], sb, wt[:, h, :],
                                         start=(st[0] and h == 0),
                                         stop=(st[1] and h == 3),
                                         skip_group_check=True)
                    for pp in range(2):
                        sq = xqr_sb[:, 4 * j + 2 * pp:4 * j + 2 * pp + 2, :]
                        nc.tensor.matmul(uP[ph][:], sq,
                                         dt_[:, 2 * pp:2 * pp + 2, :],
                                         perf_mode=DR,
                                         start=(st[0] and pp == 0),
                                         stop=(st[1] and pp == 1),
                                         skip_group_check=True)


                # per-half tail: h1, g, gp, g-transposes, partial feats,
                # U-fold, z1, z1-transposes. For ph=0 this overlaps the
                # right half's stream.
                nc.vector.tensor_add(h1c[:, co:co + HF], hP[ph][:],
                                     bc_sb[0:B, BC_B1 + co:BC_B1 + co + HF])
                nc.scalar.activation(g_c[:, co:co + HF], h1c[:, co:co + HF],
                                     AF.Gelu_apprx_tanh)
                nc.scalar.activation(gp_bt[:, co:co + HF],
                                     h1c[:, co:co + HF], AF.Derivative_Gelu)
                for fo in range(KH * ph, KH * (ph + 1)):
                    tp = pss.tile([128, B], F32, name="ps", tag="ps")
                    nc.tensor.transpose(tp[:],
                                        g_c[:, fo * 128:(fo + 1) * 128],
                                        eye16)
                    nc.vector.tensor_copy(gT_b[:, fo * B:(fo + 1) * B],
                                          tp[:])
                for k in range(KH * ph, KH * (ph + 1)):
                    nc.tensor.matmul(fa[:], gT_b[:, k * B:(k + 1) * B],
                                     w2b_sb[:, k * F:k * F + HF],
                                     start=(k == 0), stop=(k == KF - 1),
                                     skip_group_check=True)
                    nc.tensor.matmul(fb[:], gT_b[:, k * B:(k + 1) * B],
                                     w2b_sb[:, k * F + HF:(k + 1) * F],
                                     start=(k == 0), stop=(k == KF - 1),
                                     skip_group_check=True)
                nc.vector.tensor_scalar(cmbu[0:32, co:co + HF], uP[ph][:],
                                        1.0 / WS, None, OP.mult)
                uf = pss.tile([B, HF], F32, name="ps", tag="ps")
                nc.tensor.matmul(uf[:], fsel_sb[:], cmbu[:, co:co + HF],
                                 start=True, stop=True,
                                 skip_group_check=True)
                nc.vector.tensor_mul(z1c[:, co:co + HF], uf[:],
                                     gp_bt[:, co:co + HF])
                for fo in range(KH * ph, KH * (ph + 1)):
                    tp = pss.tile([128, B], F32, name="ps", tag="ps")
                    nc.tensor.transpose(tp[:],
                                        z1c[:, fo * 128:(fo + 1) * 128],
                                        eye16)
                    nc.vector.tensor_copy(z1T[:, fo * B:(fo + 1) * B],
                                          tp[:])

            feats = wrk.tile([B, F], F32)
            nc.vector.tensor_add(feats[:, 0:HF], fa[:], bcs(BC_B2, HF))
            nc.vector.tensor_add(feats[:, HF:F], fb[:],
                                 bc_sb[0:B, BC_B2 + HF:BC_B2 + F])
            # ---- feats = g @ W2 + b2 ; V = g @ dW2 (independent) ----
            fmask = wrk.tile([B, F], F32)
            nc.vector.tensor_scalar(fmask[:], feats[:],
                                    bc_sb[0:B, BC_FB:BC_FB + 1],
                                    None, OP.mult)

            featsT = wrk.tile([128, KF * B], BF16)
            for fo in range(KF):
                tp = pss.tile([128, B], F32, name="ps", tag="ps")
                nc.tensor.transpose(tp[:], feats[:, fo * 128:(fo + 1) * 128],
                                    eye16)
                nc.vector.tensor_copy(featsT[:, fo * B:(fo + 1) * B], tp[:])

            # ---- meta-net -> coefs -> csel ----
            mps = pss.tile([B, HID], F32, name="ps", tag="ps")
            for k in range(KF):
                nc.tensor.matmul(mps[:], featsT[:, k * B:(k + 1) * B],
                                 mw1t_sb[:, k * HID:(k + 1) * HID],
                                 start=(k == 0), stop=(k == KF - 1),
                                 skip_group_check=True)
            mh_bt = wrk.tile([B, HID], F32)
            mtmp = wrk.tile([B, HID], F32)
            nc.vector.tensor_add(mtmp[:], mps[:], bcs(BC_MB1, HID))
            nc.vector.tensor_relu(mh_bt[:], mtmp[:])

            mh_tr = wrk.tile([128, 2 * B], BF16)
            tp = pss.tile([128, B], F32, name="ps", tag="ps")
            nc.tensor.transpose(tp[:], mh_bt[:, 0:128], eye16)
            nc.vector.tensor_copy(mh_tr[:, 0:B], tp[:])
            tp = pss.tile([128, B], F32, name="ps", tag="ps")
            nc.tensor.transpose(tp[0:HID - 128, :], mh_bt[:, 128:HID], eye16)
            nc.vector.tensor_copy(mh_tr[0:HID - 128, B:2 * B],
                                  tp[0:HID - 128, :])

            cps = pss.tile([B, T], F32, name="ps", tag="ps")
            nc.tensor.matmul(cps[:], mh_tr[:, 0:B], mw2t_sb[:, 0:T],
                             start=True, stop=False, skip_group_check=True)
            nc.tensor.matmul(cps[:], mh_tr[0:HID - 128, B:2 * B],
                             mw2t_sb[0:HID - 128, T:2 * T],
                             start=False, stop=True, skip_group_check=True)
            coefs_bt = wrk.tile([B, T], F32)
            nc.vector.tensor_add(coefs_bt[:], cps[:], bcs(BC_MB2, T))
            csel = wrk.tile([B, 1], F32)
            cjunk = wrk.tile([B, T], F32)
            nc.vector.tensor_mul(cjunk[:], coefs_bt[:], bcs(BC_SEL, T))
            nc.vector.reduce_sum(csel[:], cjunk[:], axis=mybir.AxisListType.X)

            # ---- V = g @ dW2 + db2 ----
            v5 = pss.tile([B, 512], F32, name="ps", tag="ps")
            v2 = pss.tile([B, 256], F32, name="ps", tag="ps")
            for k in range(KF):
                nc.tensor.matmul(v5[:], gT_b[:, k * B:(k + 1) * B],
                                 dw2b_sb[:, k * F:k * F + 512],
                                 start=(k == 0), stop=(k == KF - 1),
                                 skip_group_check=True)
                nc.tensor.matmul(v2[:], gT_b[:, k * B:(k + 1) * B],
                                 dw2b_sb[:, k * F + 512:(k + 1) * F],
                                 start=(k == 0), stop=(k == KF - 1),
                                 skip_group_check=True)
            v_bt = wrk.tile([B, F], F32)
            nc.vector.tensor_add(v_bt[:, 0:512], v5[:], bcs(BC_DB2, 512))
            nc.vector.tensor_add(v_bt[:, 512:F], v2[:],
                                 bc_sb[0:B, BC_DB2 + 512:BC_DB2 + F])

            # ---- dout1 = z1 @ W2 ; ot = fb*feats + csel*(dout1 + V) ----
            o5 = pss.tile([B, 512], F32, name="ps", tag="ps")
            o2 = pss.tile([B, 256], F32, name="ps", tag="ps")
            for k in range(KF):
                nc.tensor.matmul(o5[:], z1T[:, k * B:(k + 1) * B],
                                 w2b_sb[:, k * F:k * F + 512],
                                 start=(k == 0), stop=(k == KF - 1),
                                 skip_group_check=True)
                nc.tensor.matmul(o2[:], z1T[:, k * B:(k + 1) * B],
                                 w2b_sb[:, k * F + 512:(k + 1) * F],
                                 start=(k == 0), stop=(k == KF - 1),
                                 skip_group_check=True)
            od = wrk.tile([B, F], F32)
            nc.vector.tensor_add(od[:, 0:512], o5[:], v_bt[:, 0:512])
            nc.vector.tensor_add(od[:, 512:F], o2[:], v_bt[:, 512:F])
            od2 = wrk.tile([B, F], F32)
            nc.vector.tensor_scalar(od2[:], od[:], csel[:], None, OP.mult)
            out2 = wrk.tile([B, F], F32)
            nc.vector.tensor_add(out2[:], od2[:], fmask[:])
            nc.scalar.dma_start(OT.ap(), out2[:])

    nc.compile()
    return nc


def _get_nc():
    if "nc" not in _CACHE:
        _CACHE["nc"] = build()
    return _CACHE["nc"]


def _prep_in_maps(x, W1, b1, W2, b2, mW1, mb1, mW2, mb2, dW1, db1, dW2, db2):
    f32 = np.float32
    bf16 = ml_dtypes.bfloat16
    fp8 = ml_dtypes.float8_e4m3
    X = np.ascontiguousarray(np.asarray(x, f32).reshape(B, D))
    XT = np.ascontiguousarray(X.T)                       # [D, B]
    xt3 = np.ascontiguousarray(
        XT.reshape(KD, 128, B).transpose(1, 0, 2))       # [128, KD, B]
    xq = xt3.astype(fp8)
    xr = ((xt3 - xq.astype(f32)) * RS).astype(fp8)
    xqr = np.empty((128, KD, 32), fp8)
    xqr[:, :, 0:B] = xq
    xqr[:, :, B:32] = xr
    xtb = xt3.astype(bf16)
    W1 = np.asarray(W1, f32)
    W2 = np.asarray(W2, f32)
    b1 = np.asarray(b1, f32); b2 = np.asarray(b2, f32)
    mb1 = np.asarray(mb1, f32); mb2 = np.asarray(mb2, f32)
    dW1 = np.asarray(dW1, f32); db1 = np.asarray(db1, f32)
    dW2 = np.asarray(dW2, f32); db2 = np.asarray(db2, f32)

    def pairs(w):  # [D, F] -> [KJ*128, 2, F]
        return np.ascontiguousarray(
            w.reshape(KJ, 2, 128, F).transpose(0, 2, 1, 3)
            .reshape(KJ * 128, 2, F))

    def pairs4(w):  # [D, F] -> [JJ*128, 4, F]
        return np.ascontiguousarray(
            w.reshape(JJ, 4, 128, F).transpose(0, 2, 1, 3)
            .reshape(JJ * 128, 4, F))

    w1p = pairs4(W1).astype(bf16)
    w1l = np.ascontiguousarray(w1p[:, :, 0:HF])
    w1rr = np.ascontiguousarray(w1p[:, :, HF:F])
    w2b = np.ascontiguousarray(W2).astype(bf16)
    mw1t = np.ascontiguousarray(
        np.asarray(mW1, f32).T.reshape(KF, 128, HID).transpose(1, 0, 2)
        .reshape(128, KF * HID)).astype(bf16)
    mw2tf = np.asarray(mW2, f32).T                       # [HID, T]
    mw2t = np.zeros((128, 2 * T), f32)
    mw2t[:, 0:T] = mw2tf[0:128, :]
    mw2t[0:HID - 128, T:2 * T] = mw2tf[128:HID, :]
    mw2t = mw2t.astype(bf16)
    fsel = np.zeros((33, B), f32)
    for b in range(B):
        fsel[b, b] = 1.0
        fsel[B + b, b] = 1.0 / RS
    fsel[32, :] = 1.0

    in_maps = []
    for c in range(NCORES):
        dwp = (pairs4(dW1[c]) * WS).astype(fp8)
        bc = np.zeros((32, BCW), f32)
        bc[0:B, BC_B1:BC_B1 + F] = b1[None, :]
        bc[0:B, BC_B2:BC_B2 + F] = b2[None, :]
        bc[0:B, BC_DB2:BC_DB2 + F] = db2[c][None, :]
        bc[0:B, BC_MB1:BC_MB1 + HID] = mb1[None, :]
        bc[0:B, BC_MB2:BC_MB2 + T] = mb2[None, :]
        bc[0:B, BC_SEL + c] = 1.0
        if c == 0:
            bc[0:B, BC_FB] = 1.0
        bc[0:32, BC_EYE:BC_EYE + 32] = np.eye(32, dtype=f32)
        in_maps.append({
            "xqr": xqr,
            "xtb": xtb,
            "w1l": w1l,
            "w1r": w1rr,
            "dw1l": np.ascontiguousarray(dwp[:, :, 0:HF]),
            "dw1r": np.ascontiguousarray(dwp[:, :, HF:F]),
            "w2b": w2b,
            "dw2b": np.ascontiguousarray(dW2[c]).astype(bf16),
            "mw1t": mw1t,
            "mw2t": mw2t,
            "fsel": fsel,
            "db1r": db1[c][None, :],
            "bc": bc,
        })
    return in_maps


def run(inputs, trace=False, trace_cores=None, tmpdir=None):
    nc = _get_nc()
    in_maps = _prep_in_maps(**inputs)
    res = bass_utils.run_bass_kernel_spmd(
        nc, in_maps, core_ids=list(range(NCORES)), trace=trace,
        trace_cores=trace_cores, tmpdir=tmpdir)
    acc = res.results[0]["ot"].astype(np.float64)
    for c in range(1, NCORES):
        acc = acc + res.results[c]["ot"].astype(np.float64)
    return acc.astype(np.float32), res


def kernel(**inputs):
    out, _ = run(inputs, trace=False)
    return out


# revision 31
# speedup vs baseline: 1.0541x; 1.0118x over previous
"""Trainium2 Bass kernel for nn_MetaNetLinearizedModel (collective-free).

Math (B=16, D=12288, F=768, HID=192, T=8):
    X = x.reshape(B, D)
    h1 = X @ W1 + b1                       [B, F]
    g  = gelu_tanh(h1); gp = gelu_tanh'(h1)
    feats = g @ W2 + b2                    [B, F]
    mh = relu(feats @ mW1.T + mb1)         [B, HID]
    coefs = mh @ mW2.T + mb2               [B, T]
    dh1_t = X @ dW1[t] + db1[t]
    dout  = sum_t coefs[:,t] * ((gp * dh1_t) @ W2 + g @ dW2[t] + db2[t])
    out   = feats + dout

Per-core plan (8 cores, NO collectives -> no cc barrier / dead window):
  - All big matmuls run fp8 DoubleRow with 2-term residual splits:
      x  = Xq + Xr/32        (Xq = fp8(x), Xr = fp8(32*(x-Xq)))
      W1 = (W1q + W1r/32)/64 (W1q = fp8(64*W1), W1r = fp8(2048*resid))
    Per k-pair, ONE stationary [128,2,32] = [Xq|Xr] feeds 6 DoubleRow
    matmuls: hA = [Xq|Xr]@W1q, hB = [Xq|Xr]@W1r, U = [Xq|Xr]@dW1q,
    each accumulating [32, 512]+[32, 256] PSUM.
  - Cross-partition folds (rows 16:32 scaled 1/32 into rows 0:16) and
    the b1/db1 bias adds happen via one [33,16] selection matmul per
    path (the 33rd row carries the bias).
    h1 ~ fold(hA) + hB[0:16]/2048  -> bf16-or-better accuracy for
    h1/g/gp/feats/coefs; delta dW1 stays 1-term fp8 (error ~9e-3 rel,
    budget 2e-2).
  - W2 / dW2 / meta-net / stationary casts bf16; g & gp via scalar
    engine activation LUTs (Gelu_apprx_tanh / Derivative_Gelu).
  - out_c = (c==0)*feats + coefs[:,c]*(z1 @ W2 + g @ dW2[c] + db2[c]),
    z1 = gp*(U + db1[c]); host sums the 8 core outputs.
"""
import sys

sys.path.insert(0, "/opt/trn_rl_repo")

import numpy as np
import ml_dtypes
import concourse.bass as bass
import concourse.bacc as bacc
import concourse.tile as tile
import concourse.mybir as mybir
from concourse import bass_utils


F32 = mybir.dt.float32
F32R = mybir.dt.float32r
BF16 = mybir.dt.bfloat16
FP8 = mybir.dt.float8e4
AF = mybir.ActivationFunctionType
OP = mybir.AluOpType
DR = mybir.MatmulPerfMode.DoubleRow

B = 16
D = 3 * 64 * 64        # 12288
F = 768
HID = 192
T = 8
NCORES = 8
KD = D // 128          # 96 k-tiles over D
KJ = KD // 2           # 48 pair-tiles
KF = F // 128          # 6
FL = 512               # left phase column width (F-split streaming)
FR = F - FL            # 256 right phase width
KL = FL // 128         # 4
JJ = KJ // 2           # 24 two-pair chunks
WS = 64.0              # fp8 weight pre-scale
RS = 32.0              # residual pre-scale (x and W1)
GELU_C0 = float(np.sqrt(2.0 / np.pi))
GELU_A = 0.044715

# bc pack column offsets (fp32 [32, BCW])
BC_B1 = 0                   # width F
BC_B2 = BC_B1 + F           # width F
BC_DB2 = BC_B2 + F          # width F;  db2[c]
BC_MB1 = BC_DB2 + F         # width HID
BC_MB2 = BC_MB1 + HID       # width T
BC_SEL = BC_MB2 + T         # width T
BC_FB = BC_SEL + T          # width 1; 1.0 on core 0 else 0.0
BC_EYE = BC_FB + 1          # width 32
BCW = BC_EYE + 32

_CACHE = {}


def build():
    nc = bacc.Bacc("TRN2", target_bir_lowering=False, debug=False,
                   enable_asserts=False, num_devices=NCORES)

    XQR = nc.dram_tensor("xqr", [128, KD, 32], FP8, kind="ExternalInput")
    XTB = nc.dram_tensor("xtb", [128, KD, B], BF16, kind="ExternalInput")
    W1L = nc.dram_tensor("w1l", [JJ * 128, 4, FL], BF16, kind="ExternalInput")
    W1R = nc.dram_tensor("w1r", [JJ * 128, 4, FR], BF16, kind="ExternalInput")
    DW1L = nc.dram_tensor("dw1l", [JJ * 128, 4, FL], FP8, kind="ExternalInput")
    DW1R = nc.dram_tensor("dw1r", [JJ * 128, 4, FR], FP8, kind="ExternalInput")
    W2B = nc.dram_tensor("w2b", [F, F], BF16, kind="ExternalInput")
    DW2B = nc.dram_tensor("dw2b", [F, F], BF16, kind="ExternalInput")
    MW1T = nc.dram_tensor("mw1t", [128, KF * HID], BF16, kind="ExternalInput")
    MW2T = nc.dram_tensor("mw2t", [128, 2 * T], BF16, kind="ExternalInput")
    FSEL = nc.dram_tensor("fsel", [33, B], F32R, kind="ExternalInput")
    DB1R = nc.dram_tensor("db1r", [1, F], F32R, kind="ExternalInput")
    BC = nc.dram_tensor("bc", [32, BCW], F32, kind="ExternalInput")
    OT = nc.dram_tensor("ot", [B, F], F32, kind="ExternalOutput")

    with tile.TileContext(nc, num_cores=NCORES) as tc:
        with (
            tc.tile_pool(name="cst", bufs=1) as cst,
            tc.tile_pool(name="wrk", bufs=1) as wrk,
            tc.tile_pool(name="w1br", bufs=12) as w1br,
            tc.tile_pool(name="dw1p", bufs=12) as dw1p,
            tc.tile_pool(name="psu", bufs=1, space="PSUM") as psu,
            tc.tile_pool(name="pss", bufs=2, space="PSUM") as pss,
        ):
            # ---- priority loads (scalar queue) ----
            xqr_sb = cst.tile([128, KD, 32], FP8)
            nc.scalar.dma_start(xqr_sb[:], XQR.ap())
            xtb_sb = cst.tile([128, KD, B], BF16)
            nc.scalar.dma_start(xtb_sb[:], XTB.ap())
            bc_sb = cst.tile([32, BCW], F32)
            nc.scalar.dma_start(bc_sb[:], BC.ap())
            fsel_sb = cst.tile([33, B], F32R)
            nc.scalar.dma_start(fsel_sb[:], FSEL.ap())
            cmbu = wrk.tile([33, F], F32R)
            nc.scalar.dma_start(cmbu[32:33, :], DB1R.ap())
            # preload gelu table while the scalar engine is idle
            scr = wrk.tile([1, 8], F32)
            nc.scalar.activation(scr[0:1, 0:1], bc_sb[0:1, 0:1],
                                 AF.Gelu_apprx_tanh)
            # tail loads (consumed only after ~half the stream)
            w2b_sb = cst.tile([128, KF * F], BF16)
            for k in range(KF):
                nc.scalar.dma_start(w2b_sb[:, k * F:(k + 1) * F],
                                    W2B.ap()[k * 128:(k + 1) * 128, :])
            mw1t_sb = cst.tile([128, KF * HID], BF16)
            nc.scalar.dma_start(mw1t_sb[:], MW1T.ap())
            mw2t_sb = cst.tile([128, 2 * T], BF16)
            nc.scalar.dma_start(mw2t_sb[:], MW2T.ap())
            dw2b_sb = cst.tile([128, KF * F], BF16)
            for k in range(KF):
                nc.scalar.dma_start(dw2b_sb[:, k * F:(k + 1) * F],
                                    DW2B.ap()[k * 128:(k + 1) * 128, :])

            def bcs(col, w):
                return bc_sb[0:B, col:col + w]
            eye16 = bc_sb[0:B, BC_EYE:BC_EYE + B]

            # ---- main stream (F-split): bf16 h-chains + fp8 DoubleRow U.
            # Left half-columns stream first; the left tail (h1/gelu/
            # transposes/feats/z1) overlaps the right half's stream. ----
            hP = [psu.tile([B, FL], F32, name="hL"),
                  psu.tile([B, FR], F32, name="hR")]
            uP = [psu.tile([32, FL], F32, name="uL"),
                  psu.tile([32, FR], F32, name="uR")]
            h1c = wrk.tile([B, F], F32)
            g_c = wrk.tile([B, F], F32)
            gp_bt = wrk.tile([B, F], F32)
            gT_b = wrk.tile([128, KF * B], BF16)
            z1c = wrk.tile([B, F], F32)
            z1T = wrk.tile([128, KF * B], BF16)
            fa = psu.tile([B, FL], F32, name="fa")
            fb = psu.tile([B, FR], F32, name="fb")

            for ph, (W1X, DW1X, co, cw) in enumerate(
                    ((W1L, DW1L, 0, FL), (W1R, DW1R, FL, FR))):
                for j in range(JJ):
                    wt = w1br.tile([128, 4, cw], BF16, name="w1t",
                                   tag=f"w1t{ph}")
                    dt_ = dw1p.tile([128, 4, cw], FP8, name="dw1t",
                                    tag=f"dw1t{ph}")
                    if j % 2 == 0:
                        nc.sync.dma_start(wt[:],
                                          W1X.ap()[j * 128:(j + 1) * 128])
                        nc.gpsimd.dma_start(dt_[:],
                                            DW1X.ap()[j * 128:(j + 1) * 128])
                    else:
                        nc.gpsimd.dma_start(wt[:],
                                            W1X.ap()[j * 128:(j + 1) * 128])
                        nc.sync.dma_start(dt_[:],
                                          DW1X.ap()[j * 128:(j + 1) * 128])
                    st = (j == 0), (j == JJ - 1)
                    for h in range(4):
                        sb = xtb_sb[:, 4 * j + h, :]
                        nc.tensor.matmul(hP[ph][:], sb, wt[:, h, :],
                                         start=(st[0] and h == 0),
                                         stop=(st[1] and h == 3),
                                         skip_group_check=True)
                    for pp in range(2):
                        sq = xqr_sb[:, 4 * j + 2 * pp:4 * j + 2 * pp + 2, :]
                        nc.tensor.matmul(uP[ph][:], sq,
                                         dt_[:, 2 * pp:2 * pp + 2, :],
                                         perf_mode=DR,
                                         start=(st[0] and pp == 0),
                                         stop=(st[1] and pp == 1),
                                         skip_group_check=True)

                # per-half tail: h1, g, gp, g-transposes, partial feats,
                # U-fold, z1, z1-transposes. For ph=0 this overlaps the
                # right phase's stream.
                nc.vector.tensor_add(h1c[:, co:co + cw], hP[ph][:],
                                     bc_sb[0:B, BC_B1 + co:BC_B1 + co + cw])
                nc.scalar.activation(g_c[:, co:co + cw], h1c[:, co:co + cw],
                                     AF.Gelu_apprx_tanh)
                nc.scalar.activation(gp_bt[:, co:co + cw],
                                     h1c[:, co:co + cw], AF.Derivative_Gelu)
                for fo in range(co // 128, (co + cw) // 128):
                    tp = pss.tile([128, B], F32, name="ps", tag="ps")
                    nc.tensor.transpose(tp[:],
                                        g_c[:, fo * 128:(fo + 1) * 128],
                                        eye16)
                    nc.vector.tensor_copy(gT_b[:, fo * B:(fo + 1) * B],
                                          tp[:])
                for k in range(co // 128, (co + cw) // 128):
                    nc.tensor.matmul(fa[:], gT_b[:, k * B:(k + 1) * B],
                                     w2b_sb[:, k * F:k * F + FL],
                                     start=(k == 0), stop=(k == KF - 1),
                                     skip_group_check=True)
                    nc.tensor.matmul(fb[:], gT_b[:, k * B:(k + 1) * B],
                                     w2b_sb[:, k * F + FL:(k + 1) * F],
                                     start=(k == 0), stop=(k == KF - 1),
                                     skip_group_check=True)
                nc.vector.tensor_scalar(cmbu[0:32, co:co + cw], uP[ph][:],
                                        1.0 / WS, None, OP.mult)
                uf = pss.tile([B, cw], F32, name="ps", tag="ps")
                nc.tensor.matmul(uf[:], fsel_sb[:], cmbu[:, co:co + cw],
                                 start=True, stop=True,
                                 skip_group_check=True)
                nc.vector.tensor_mul(z1c[:, co:co + cw], uf[:],
                                     gp_bt[:, co:co + cw])
                for fo in range(co // 128, (co + cw) // 128):
                    tp = pss.tile([128, B], F32, name="ps", tag="ps")
                    nc.tensor.transpose(tp[:],
                                        z1c[:, fo * 128:(fo + 1) * 128],
                                        eye16)
                    nc.vector.tensor_copy(z1T[:, fo * B:(fo + 1) * B],
                                          tp[:])

            feats = wrk.tile([B, F], F32)
            nc.vector.tensor_add(feats[:, 0:FL], fa[:], bcs(BC_B2, FL))
            nc.vector.tensor_add(feats[:, FL:F], fb[:],
                                 bc_sb[0:B, BC_B2 + FL:BC_B2 + F])
            # ---- feats = g @ W2 + b2 ; V = g @ dW2 (independent) ----
            fmask = wrk.tile([B, F], F32)
            nc.vector.tensor_scalar(fmask[:], feats[:],
                                    bc_sb[0:B, BC_FB:BC_FB + 1],
                                    None, OP.mult)

            featsT = wrk.tile([128, KF * B], BF16)
            for fo in range(KF):
                tp = pss.tile([128, B], F32, name="ps", tag="ps")
                nc.tensor.transpose(tp[:], feats[:, fo * 128:(fo + 1) * 128],
                                    eye16)
                nc.vector.tensor_copy(featsT[:, fo * B:(fo + 1) * B], tp[:])

            # ---- meta-net -> coefs -> csel ----
            mps = pss.tile([B, HID], F32, name="ps", tag="ps")
            for k in range(KF):
                nc.tensor.matmul(mps[:], featsT[:, k * B:(k + 1) * B],
                                 mw1t_sb[:, k * HID:(k + 1) * HID],
                                 start=(k == 0), stop=(k == KF - 1),
                                 skip_group_check=True)
            mh_bt = wrk.tile([B, HID], F32)
            mtmp = wrk.tile([B, HID], F32)
            nc.vector.tensor_add(mtmp[:], mps[:], bcs(BC_MB1, HID))
            nc.vector.tensor_relu(mh_bt[:], mtmp[:])

            mh_tr = wrk.tile([128, 2 * B], BF16)
            tp = pss.tile([128, B], F32, name="ps", tag="ps")
            nc.tensor.transpose(tp[:], mh_bt[:, 0:128], eye16)
            nc.vector.tensor_copy(mh_tr[:, 0:B], tp[:])
            tp = pss.tile([128, B], F32, name="ps", tag="ps")
            nc.tensor.transpose(tp[0:HID - 128, :], mh_bt[:, 128:HID], eye16)
            nc.vector.tensor_copy(mh_tr[0:HID - 128, B:2 * B],
                                  tp[0:HID - 128, :])

            cps = pss.tile([B, T], F32, name="ps", tag="ps")
            nc.tensor.matmul(cps[:], mh_tr[:, 0:B], mw2t_sb[:, 0:T],
                             start=True, stop=False, skip_group_check=True)
            nc.tensor.matmul(cps[:], mh_tr[0:HID - 128, B:2 * B],
                             mw2t_sb[0:HID - 128, T:2 * T],
                             start=False, stop=True, skip_group_check=True)
            coefs_bt = wrk.tile([B, T], F32)
            nc.vector.tensor_add(coefs_bt[:], cps[:], bcs(BC_MB2, T))
            csel = wrk.tile([B, 1], F32)
            cjunk = wrk.tile([B, T], F32)
            nc.vector.tensor_mul(cjunk[:], coefs_bt[:], bcs(BC_SEL, T))
            nc.vector.reduce_sum(csel[:], cjunk[:], axis=mybir.AxisListType.X)

            # ---- V = g @ dW2 + db2 ----
            v5 = pss.tile([B, 512], F32, name="ps", tag="ps")
            v2 = pss.tile([B, 256], F32, name="ps", tag="ps")
            for k in range(KF):
                nc.tensor.matmul(v5[:], gT_b[:, k * B:(k + 1) * B],
                                 dw2b_sb[:, k * F:k * F + 512],
                                 start=(k == 0), stop=(k == KF - 1),
                                 skip_group_check=True)
                nc.tensor.matmul(v2[:], gT_b[:, k * B:(k + 1) * B],
                                 dw2b_sb[:, k * F + 512:(k + 1) * F],
                                 start=(k == 0), stop=(k == KF - 1),
                                 skip_group_check=True)
            v_bt = wrk.tile([B, F], F32)
            nc.vector.tensor_add(v_bt[:, 0:512], v5[:], bcs(BC_DB2, 512))
            nc.vector.tensor_add(v_bt[:, 512:F], v2[:],
                                 bc_sb[0:B, BC_DB2 + 512:BC_DB2 + F])

            # ---- dout1 = z1 @ W2 ; ot = fb*feats + csel*(dout1 + V) ----
            o5 = pss.tile([B, 512], F32, name="ps", tag="ps")
            o2 = pss.tile([B, 256], F32, name="ps", tag="ps")
            for k in range(KF):
                nc.tensor.matmul(o5[:], z1T[:, k * B:(k + 1) * B],
                                 w2b_sb[:, k * F:k * F + 512],
                                 start=(k == 0), stop=(k == KF - 1),
                                 skip_group_check=True)
                nc.tensor.matmul(o2[:], z1T[:, k * B:(k + 1) * B],
                                 w2b_sb[:, k * F + 512:(k + 1) * F],
                                 start=(k == 0), stop=(k == KF - 1),
                                 skip_group_check=True)
            od = wrk.tile([B, F], F32)
            nc.vector.tensor_add(od[:, 0:512], o5[:], v_bt[:, 0:512])
            nc.vector.tensor_add(od[:, 512:F], o2[:], v_bt[:, 512:F])
            od2 = wrk.tile([B, F], F32)
            nc.vector.tensor_scalar(od2[:], od[:], csel[:], None, OP.mult)
            out2 = wrk.tile([B, F], F32)
            nc.vector.tensor_add(out2[:], od2[:], fmask[:])
            nc.scalar.dma_start(OT.ap(), out2[:])

    nc.compile()
    return nc


def _get_nc():
    if "nc" not in _CACHE:
        _CACHE["nc"] = build()
    return _CACHE["nc"]


def _prep_in_maps(x, W1, b1, W2, b2, mW1, mb1, mW2, mb2, dW1, db1, dW2, db2):
    f32 = np.float32
    bf16 = ml_dtypes.bfloat16
    fp8 = ml_dtypes.float8_e4m3
    X = np.ascontiguousarray(np.asarray(x, f32).reshape(B, D))
    XT = np.ascontiguousarray(X.T)                       # [D, B]
    xt3 = np.ascontiguousarray(
        XT.reshape(KD, 128, B).transpose(1, 0, 2))       # [128, KD, B]
    xq = xt3.astype(fp8)
    xr = ((xt3 - xq.astype(f32)) * RS).astype(fp8)
    xqr = np.empty((128, KD, 32), fp8)
    xqr[:, :, 0:B] = xq
    xqr[:, :, B:32] = xr
    xtb = xt3.astype(bf16)
    W1 = np.asarray(W1, f32)
    W2 = np.asarray(W2, f32)
    b1 = np.asarray(b1, f32); b2 = np.asarray(b2, f32)
    mb1 = np.asarray(mb1, f32); mb2 = np.asarray(mb2, f32)
    dW1 = np.asarray(dW1, f32); db1 = np.asarray(db1, f32)
    dW2 = np.asarray(dW2, f32); db2 = np.asarray(db2, f32)

    def pairs(w):  # [D, F] -> [KJ*128, 2, F]
        return np.ascontiguousarray(
            w.reshape(KJ, 2, 128, F).transpose(0, 2, 1, 3)
            .reshape(KJ * 128, 2, F))

    def pairs4(w):  # [D, F] -> [JJ*128, 4, F]
        return np.ascontiguousarray(
            w.reshape(JJ, 4, 128, F).transpose(0, 2, 1, 3)
            .reshape(JJ * 128, 4, F))

    w1p = pairs4(W1).astype(bf16)
    w1l = np.ascontiguousarray(w1p[:, :, 0:FL])
    w1rr = np.ascontiguousarray(w1p[:, :, FL:F])
    w2b = np.ascontiguousarray(W2).astype(bf16)
    mw1t = np.ascontiguousarray(
        np.asarray(mW1, f32).T.reshape(KF, 128, HID).transpose(1, 0, 2)
        .reshape(128, KF * HID)).astype(bf16)
    mw2tf = np.asarray(mW2, f32).T                       # [HID, T]
    mw2t = np.zeros((128, 2 * T), f32)
    mw2t[:, 0:T] = mw2tf[0:128, :]
    mw2t[0:HID - 128, T:2 * T] = mw2tf[128:HID, :]
    mw2t = mw2t.astype(bf16)
    fsel = np.zeros((33, B), f32)
    for b in range(B):
        fsel[b, b] = 1.0
        fsel[B + b, b] = 1.0 / RS
    fsel[32, :] = 1.0

    in_maps = []
    for c in range(NCORES):
        dwp = (pairs4(dW1[c]) * WS).astype(fp8)
        bc = np.zeros((32, BCW), f32)
        bc[0:B, BC_B1:BC_B1 + F] = b1[None, :]
        bc[0:B, BC_B2:BC_B2 + F] = b2[None, :]
        bc[0:B, BC_DB2:BC_DB2 + F] = db2[c][None, :]
        bc[0:B, BC_MB1:BC_MB1 + HID] = mb1[None, :]
        bc[0:B, BC_MB2:BC_MB2 + T] = mb2[None, :]
        bc[0:B, BC_SEL + c] = 1.0
        if c == 0:
            bc[0:B, BC_FB] = 1.0
        bc[0:32, BC_EYE:BC_EYE + 32] = np.eye(32, dtype=f32)
        in_maps.append({
            "xqr": xqr,
            "xtb": xtb,
            "w1l": w1l,
            "w1r": w1rr,
            "dw1l": np.ascontiguousarray(dwp[:, :, 0:FL]),
            "dw1r": np.ascontiguousarray(dwp[:, :, FL:F]),
            "w2b": w2b,
            "dw2b": np.ascontiguousarray(dW2[c]).astype(bf16),
            "mw1t": mw1t,
            "mw2t": mw2t,
            "fsel": fsel,
            "db1r": db1[c][None, :],
            "bc": bc,
        })
    return in_maps


def run(inputs, trace=False, trace_cores=None, tmpdir=None):
    nc = _get_nc()
    in_maps = _prep_in_maps(**inputs)
    res = bass_utils.run_bass_kernel_spmd(
        nc, in_maps, core_ids=list(range(NCORES)), trace=trace,
        trace_cores=trace_cores, tmpdir=tmpdir)
    acc = res.results[0]["ot"].astype(np.float64)
    for c in range(1, NCORES):
        acc = acc + res.results[c]["ot"].astype(np.float64)
    return acc.astype(np.float32), res


def kernel(**inputs):
    out, _ = run(inputs, trace=False)
    return out


# revision 32
# speedup vs baseline: 1.0629x; 1.0084x over previous
"""Trainium2 Bass kernel for nn_MetaNetLinearizedModel (collective-free).

Math (B=16, D=12288, F=768, HID=192, T=8):
    X = x.reshape(B, D)
    h1 = X @ W1 + b1                       [B, F]
    g  = gelu_tanh(h1); gp = gelu_tanh'(h1)
    feats = g @ W2 + b2                    [B, F]
    mh = relu(feats @ mW1.T + mb1)         [B, HID]
    coefs = mh @ mW2.T + mb2               [B, T]
    dh1_t = X @ dW1[t] + db1[t]
    dout  = sum_t coefs[:,t] * ((gp * dh1_t) @ W2 + g @ dW2[t] + db2[t])
    out   = feats + dout

Per-core plan (8 cores, NO collectives: the cc barrier costs ~82us and
sat dead-center in the baseline's critical path):
  - Every core streams the FULL W1 in bf16 (18.9 MB) -> h1/g/gp/feats/
    coefs at bf16 accuracy. bf16 is deliberate: the PE ingests moving
    data at ~1 column/cycle regardless of dtype, so bf16 moves 2B/cycle
    vs fp8's 1B/cycle -- W1 as one bf16 stream is DMA-bound and
    PE-cheap, while fp8 2-term (same bytes) is PE-bound.
  - Delta path T-sharded: core c owns task t=c; dW1[c] streams fp8
    (9.4 MB, x64 pre-scale) with DoubleRow. The stationary packs
    [Xq|Xr] (fp8(x) and fp8(32*residual)) so U accumulates both terms
    in one [32, n] PSUM; a [33,16] f32r selection matmul folds rows
    16:32 (x1/32), applies 1/64, and adds db1 via a bias row.
  - F-split streaming: W1/dW1 stream left columns (0:512) first, then
    right (512:768). The left tail (h1, gelu LUTs, g/z1 transposes,
    partial feats, U-fold) executes under the right phase's stream;
    only the 256-wide right tail + meta-net + merge remain exposed.
  - g & gp via scalar-engine activation LUTs (Gelu_apprx_tanh matches
    jax tanh-gelu to 2e-5; Derivative_Gelu is the erf-gelu derivative,
    8.7e-4 off the tanh one -- negligible for the delta path).
  - W2 / dW2 / meta-net / stationary casts bf16.
  - out_c = (c==0)*feats + coefs[:,c]*(z1 @ W2 + g @ dW2[c] + db2[c]),
    z1 = gp*(U + db1[c]); the host sums the 8 core outputs (fp64).

Measured: ~134 us HW exec (baseline 171.8 us), rel err 1.1e-2
(gate 2e-2). DMA ~93 us at ~345 GB/s/core is the floor; PE ~86 us.
"""
import sys

sys.path.insert(0, "/opt/trn_rl_repo")

import numpy as np
import ml_dtypes
import concourse.bass as bass
import concourse.bacc as bacc
import concourse.tile as tile
import concourse.mybir as mybir
from concourse import bass_utils


F32 = mybir.dt.float32
F32R = mybir.dt.float32r
BF16 = mybir.dt.bfloat16
FP8 = mybir.dt.float8e4
AF = mybir.ActivationFunctionType
OP = mybir.AluOpType
DR = mybir.MatmulPerfMode.DoubleRow

B = 16
D = 3 * 64 * 64        # 12288
F = 768
HID = 192
T = 8
NCORES = 8
KD = D // 128          # 96 k-tiles over D
KJ = KD // 2           # 48 pair-tiles
KF = F // 128          # 6
FL = 512               # left phase column width (F-split streaming)
FR = F - FL            # 256 right phase width
KL = FL // 128         # 4
JJ = KJ // 2           # 24 two-pair chunks
WS = 64.0              # fp8 weight pre-scale
RS = 32.0              # residual pre-scale (x and W1)
GELU_C0 = float(np.sqrt(2.0 / np.pi))
GELU_A = 0.044715

# bc pack column offsets (fp32 [32, BCW])
BC_B1 = 0                   # width F
BC_B2 = BC_B1 + F           # width F
BC_DB2 = BC_B2 + F          # width F;  db2[c]
BC_MB1 = BC_DB2 + F         # width HID
BC_MB2 = BC_MB1 + HID       # width T
BC_SEL = BC_MB2 + T         # width T
BC_FB = BC_SEL + T          # width 1; 1.0 on core 0 else 0.0
BC_EYE = BC_FB + 1          # width 32
BCW = BC_EYE + 32

_CACHE = {}


def build():
    nc = bacc.Bacc("TRN2", target_bir_lowering=False, debug=False,
                   enable_asserts=False, num_devices=NCORES)

    XQR = nc.dram_tensor("xqr", [128, KD, 32], FP8, kind="ExternalInput")
    XTB = nc.dram_tensor("xtb", [128, KD, B], BF16, kind="ExternalInput")
    W1L = nc.dram_tensor("w1l", [JJ * 128, 4, FL], BF16, kind="ExternalInput")
    W1R = nc.dram_tensor("w1r", [JJ * 128, 4, FR], BF16, kind="ExternalInput")
    DW1L = nc.dram_tensor("dw1l", [JJ * 128, 4, FL], FP8, kind="ExternalInput")
    DW1R = nc.dram_tensor("dw1r", [JJ * 128, 4, FR], FP8, kind="ExternalInput")
    W2B = nc.dram_tensor("w2b", [F, F], BF16, kind="ExternalInput")
    DW2B = nc.dram_tensor("dw2b", [F, F], BF16, kind="ExternalInput")
    MW1T = nc.dram_tensor("mw1t", [128, KF * HID], BF16, kind="ExternalInput")
    MW2T = nc.dram_tensor("mw2t", [128, 2 * T], BF16, kind="ExternalInput")
    FSEL = nc.dram_tensor("fsel", [33, B], F32R, kind="ExternalInput")
    DB1R = nc.dram_tensor("db1r", [1, F], F32R, kind="ExternalInput")
    BC = nc.dram_tensor("bc", [32, BCW], F32, kind="ExternalInput")
    OT = nc.dram_tensor("ot", [B, F], F32, kind="ExternalOutput")

    with tile.TileContext(nc, num_cores=NCORES) as tc:
        with (
            tc.tile_pool(name="cst", bufs=1) as cst,
            tc.tile_pool(name="wrk", bufs=1) as wrk,
            tc.tile_pool(name="w1br", bufs=12) as w1br,
            tc.tile_pool(name="dw1p", bufs=12) as dw1p,
            tc.tile_pool(name="psu", bufs=1, space="PSUM") as psu,
            tc.tile_pool(name="pss", bufs=2, space="PSUM") as pss,
        ):
            # ---- priority loads (scalar queue) ----
            xqr_sb = cst.tile([128, KD, 32], FP8)
            nc.scalar.dma_start(xqr_sb[:], XQR.ap())
            xtb_sb = cst.tile([128, KD, B], BF16)
            nc.scalar.dma_start(xtb_sb[:], XTB.ap())
            bc_sb = cst.tile([32, BCW], F32)
            nc.scalar.dma_start(bc_sb[:], BC.ap())
            fsel_sb = cst.tile([33, B], F32R)
            nc.scalar.dma_start(fsel_sb[:], FSEL.ap())
            cmbu = wrk.tile([33, F], F32R)
            nc.scalar.dma_start(cmbu[32:33, :], DB1R.ap())
            # preload gelu table while the scalar engine is idle
            scr = wrk.tile([1, 8], F32)
            nc.scalar.activation(scr[0:1, 0:1], bc_sb[0:1, 0:1],
                                 AF.Gelu_apprx_tanh)
            # tail loads (consumed only after ~half the stream)
            w2b_sb = cst.tile([128, KF * F], BF16)
            for k in range(KF):
                nc.scalar.dma_start(w2b_sb[:, k * F:(k + 1) * F],
                                    W2B.ap()[k * 128:(k + 1) * 128, :])
            mw1t_sb = cst.tile([128, KF * HID], BF16)
            nc.scalar.dma_start(mw1t_sb[:], MW1T.ap())
            mw2t_sb = cst.tile([128, 2 * T], BF16)
            nc.scalar.dma_start(mw2t_sb[:], MW2T.ap())
            dw2b_sb = cst.tile([128, KF * F], BF16)
            for k in range(KF):
                nc.scalar.dma_start(dw2b_sb[:, k * F:(k + 1) * F],
                                    DW2B.ap()[k * 128:(k + 1) * 128, :])

            def bcs(col, w):
                return bc_sb[0:B, col:col + w]
            eye16 = bc_sb[0:B, BC_EYE:BC_EYE + B]

            # ---- main stream (F-split): bf16 h-chains + fp8 DoubleRow U.
            # Left half-columns stream first; the left tail (h1/gelu/
            # transposes/feats/z1) overlaps the right half's stream. ----
            hP = [psu.tile([B, FL], F32, name="hL"),
                  psu.tile([B, FR], F32, name="hR")]
            uP = [psu.tile([32, FL], F32, name="uL"),
                  psu.tile([32, FR], F32, name="uR")]
            h1c = wrk.tile([B, F], F32)
            g_c = wrk.tile([B, F], F32)
            gp_bt = wrk.tile([B, F], F32)
            gT_b = wrk.tile([128, KF * B], BF16)
            z1c = wrk.tile([B, F], F32)
            z1T = wrk.tile([128, KF * B], BF16)
            fa = psu.tile([B, FL], F32, name="fa")
            fb = psu.tile([B, FR], F32, name="fb")

            for ph, (W1X, DW1X, co, cw) in enumerate(
                    ((W1L, DW1L, 0, FL), (W1R, DW1R, FL, FR))):
                for j in range(JJ):
                    wt = w1br.tile([128, 4, cw], BF16, name="w1t",
                                   tag=f"w1t{ph}")
                    dt_ = dw1p.tile([128, 4, cw], FP8, name="dw1t",
                                    tag=f"dw1t{ph}")
                    if j % 2 == 0:
                        nc.sync.dma_start(wt[:],
                                          W1X.ap()[j * 128:(j + 1) * 128])
                        nc.gpsimd.dma_start(dt_[:],
                                            DW1X.ap()[j * 128:(j + 1) * 128])
                    else:
                        nc.gpsimd.dma_start(wt[:],
                                            W1X.ap()[j * 128:(j + 1) * 128])
                        nc.sync.dma_start(dt_[:],
                                          DW1X.ap()[j * 128:(j + 1) * 128])
                    st = (j == 0), (j == JJ - 1)
                    for h in range(4):
                        sb = xtb_sb[:, 4 * j + h, :]
                        nc.tensor.matmul(hP[ph][:], sb, wt[:, h, :],
                                         start=(st[0] and h == 0),
                                         stop=(st[1] and h == 3),
                                         skip_group_check=True)
                    for pp in range(2):
                        sq = xqr_sb[:, 4 * j + 2 * pp:4 * j + 2 * pp + 2, :]
                        nc.tensor.matmul(uP[ph][:], sq,
                                         dt_[:, 2 * pp:2 * pp + 2, :],
                                         perf_mode=DR,
                                         start=(st[0] and pp == 0),
                                         stop=(st[1] and pp == 1),
                                         skip_group_check=True)

                # per-half tail: h1, g, gp, g-transposes, partial feats,
                # U-fold, z1, z1-transposes. For ph=0 this overlaps the
                # right phase's stream.
                nc.vector.tensor_add(h1c[:, co:co + cw], hP[ph][:],
                                     bc_sb[0:B, BC_B1 + co:BC_B1 + co + cw])
                nc.scalar.activation(g_c[:, co:co + cw], h1c[:, co:co + cw],
                                     AF.Gelu_apprx_tanh)
                nc.scalar.activation(gp_bt[:, co:co + cw],
                                     h1c[:, co:co + cw], AF.Derivative_Gelu)
                for fo in range(co // 128, (co + cw) // 128):
                    tp = pss.tile([128, B], F32, name="ps", tag="ps")
                    nc.tensor.transpose(tp[:],
                                        g_c[:, fo * 128:(fo + 1) * 128],
                                        eye16)
                    nc.vector.tensor_copy(gT_b[:, fo * B:(fo + 1) * B],
                                          tp[:])
                for k in range(co // 128, (co + cw) // 128):
                    nc.tensor.matmul(fa[:], gT_b[:, k * B:(k + 1) * B],
                                     w2b_sb[:, k * F:k * F + FL],
                                     start=(k == 0), stop=(k == KF - 1),
                                     skip_group_check=True)
                    nc.tensor.matmul(fb[:], gT_b[:, k * B:(k + 1) * B],
                                     w2b_sb[:, k * F + FL:(k + 1) * F],
                                     start=(k == 0), stop=(k == KF - 1),
                                     skip_group_check=True)
                nc.vector.tensor_scalar(cmbu[0:32, co:co + cw], uP[ph][:],
                                        1.0 / WS, None, OP.mult)
                uf = pss.tile([B, cw], F32, name="ps", tag="ps")
                nc.tensor.matmul(uf[:], fsel_sb[:], cmbu[:, co:co + cw],
                                 start=True, stop=True,
                                 skip_group_check=True)
                nc.vector.tensor_mul(z1c[:, co:co + cw], uf[:],
                                     gp_bt[:, co:co + cw])
                for fo in range(co // 128, (co + cw) // 128):
                    tp = pss.tile([128, B], F32, name="ps", tag="ps")
                    nc.tensor.transpose(tp[:],
                                        z1c[:, fo * 128:(fo + 1) * 128],
                                        eye16)
                    nc.vector.tensor_copy(z1T[:, fo * B:(fo + 1) * B],
                                          tp[:])

            feats = wrk.tile([B, F], F32)
            nc.vector.tensor_add(feats[:, 0:FL], fa[:], bcs(BC_B2, FL))
            nc.vector.tensor_add(feats[:, FL:F], fb[:],
                                 bc_sb[0:B, BC_B2 + FL:BC_B2 + F])
            # ---- feats = g @ W2 + b2 ; V = g @ dW2 (independent) ----
            fmask = wrk.tile([B, F], F32)
            nc.vector.tensor_scalar(fmask[:], feats[:],
                                    bc_sb[0:B, BC_FB:BC_FB + 1],
                                    None, OP.mult)

            featsT = wrk.tile([128, KF * B], BF16)
            for fo in range(KF):
                tp = pss.tile([128, B], F32, name="ps", tag="ps")
                nc.tensor.transpose(tp[:], feats[:, fo * 128:(fo + 1) * 128],
                                    eye16)
                nc.vector.tensor_copy(featsT[:, fo * B:(fo + 1) * B], tp[:])

            # ---- meta-net -> coefs -> csel ----
            mps = pss.tile([B, HID], F32, name="ps", tag="ps")
            for k in range(KF):
                nc.tensor.matmul(mps[:], featsT[:, k * B:(k + 1) * B],
                                 mw1t_sb[:, k * HID:(k + 1) * HID],
                                 start=(k == 0), stop=(k == KF - 1),
                                 skip_group_check=True)
            mh_bt = wrk.tile([B, HID], F32)
            mtmp = wrk.tile([B, HID], F32)
            nc.vector.tensor_add(mtmp[:], mps[:], bcs(BC_MB1, HID))
            nc.vector.tensor_relu(mh_bt[:], mtmp[:])

            mh_tr = wrk.tile([128, 2 * B], BF16)
            tp = pss.tile([128, B], F32, name="ps", tag="ps")
            nc.tensor.transpose(tp[:], mh_bt[:, 0:128], eye16)
            nc.vector.tensor_copy(mh_tr[:, 0:B], tp[:])
            tp = pss.tile([128, B], F32, name="ps", tag="ps")
            nc.tensor.transpose(tp[0:HID - 128, :], mh_bt[:, 128:HID], eye16)
            nc.vector.tensor_copy(mh_tr[0:HID - 128, B:2 * B],
                                  tp[0:HID - 128, :])

            cps = pss.tile([B, T], F32, name="ps", tag="ps")
            nc.tensor.matmul(cps[:], mh_tr[:, 0:B], mw2t_sb[:, 0:T],
                             start=True, stop=False, skip_group_check=True)
            nc.tensor.matmul(cps[:], mh_tr[0:HID - 128, B:2 * B],
                             mw2t_sb[0:HID - 128, T:2 * T],
                             start=False, stop=True, skip_group_check=True)
            coefs_bt = wrk.tile([B, T], F32)
            nc.vector.tensor_add(coefs_bt[:], cps[:], bcs(BC_MB2, T))
            csel = wrk.tile([B, 1], F32)
            cjunk = wrk.tile([B, T], F32)
            nc.vector.tensor_mul(cjunk[:], coefs_bt[:], bcs(BC_SEL, T))
            nc.vector.reduce_sum(csel[:], cjunk[:], axis=mybir.AxisListType.X)

            # ---- V = g @ dW2 + db2 ----
            v5 = pss.tile([B, 512], F32, name="ps", tag="ps")
            v2 = pss.tile([B, 256], F32, name="ps", tag="ps")
            for k in range(KF):
                nc.tensor.matmul(v5[:], gT_b[:, k * B:(k + 1) * B],
                                 dw2b_sb[:, k * F:k * F + 512],
                                 start=(k == 0), stop=(k == KF - 1),
                                 skip_group_check=True)
                nc.tensor.matmul(v2[:], gT_b[:, k * B:(k + 1) * B],
                                 dw2b_sb[:, k * F + 512:(k + 1) * F],
                                 start=(k == 0), stop=(k == KF - 1),
                                 skip_group_check=True)
            v_bt = wrk.tile([B, F], F32)
            nc.vector.tensor_add(v_bt[:, 0:512], v5[:], bcs(BC_DB2, 512))
            nc.vector.tensor_add(v_bt[:, 512:F], v2[:],
                                 bc_sb[0:B, BC_DB2 + 512:BC_DB2 + F])

            # ---- dout1 = z1 @ W2 ; ot = fb*feats + csel*(dout1 + V) ----
            o5 = pss.tile([B, 512], F32, name="ps", tag="ps")
            o2 = pss.tile([B, 256], F32, name="ps", tag="ps")
            for k in range(KF):
                nc.tensor.matmul(o5[:], z1T[:, k * B:(k + 1) * B],
                                 w2b_sb[:, k * F:k * F + 512],
                                 start=(k == 0), stop=(k == KF - 1),
                                 skip_group_check=True)
                nc.tensor.matmul(o2[:], z1T[:, k * B:(k + 1) * B],
                                 w2b_sb[:, k * F + 512:(k + 1) * F],
                                 start=(k == 0), stop=(k == KF - 1),
                                 skip_group_check=True)
            od = wrk.tile([B, F], F32)
            nc.vector.tensor_add(od[:, 0:512], o5[:], v_bt[:, 0:512])
            nc.vector.tensor_add(od[:, 512:F], o2[:], v_bt[:, 512:F])
            od2 = wrk.tile([B, F], F32)
            nc.vector.tensor_scalar(od2[:], od[:], csel[:], None, OP.mult)
            out2 = wrk.tile([B, F], F32)
            nc.vector.tensor_add(out2[:], od2[:], fmask[:])
            nc.scalar.dma_start(OT.ap(), out2[:])

    nc.compile()
    return nc


def _get_nc():
    if "nc" not in _CACHE:
        _CACHE["nc"] = build()
    return _CACHE["nc"]


def _prep_in_maps(x, W1, b1, W2, b2, mW1, mb1, mW2, mb2, dW1, db1, dW2, db2):
    f32 = np.float32
    bf16 = ml_dtypes.bfloat16
    fp8 = ml_dtypes.float8_e4m3
    X = np.ascontiguousarray(np.asarray(x, f32).reshape(B, D))
    XT = np.ascontiguousarray(X.T)                       # [D, B]
    xt3 = np.ascontiguousarray(
        XT.reshape(KD, 128, B).transpose(1, 0, 2))       # [128, KD, B]
    xq = xt3.astype(fp8)
    xr = ((xt3 - xq.astype(f32)) * RS).astype(fp8)
    xqr = np.empty((128, KD, 32), fp8)
    xqr[:, :, 0:B] = xq
    xqr[:, :, B:32] = xr
    xtb = xt3.astype(bf16)
    W1 = np.asarray(W1, f32)
    W2 = np.asarray(W2, f32)
    b1 = np.asarray(b1, f32); b2 = np.asarray(b2, f32)
    mb1 = np.asarray(mb1, f32); mb2 = np.asarray(mb2, f32)
    dW1 = np.asarray(dW1, f32); db1 = np.asarray(db1, f32)
    dW2 = np.asarray(dW2, f32); db2 = np.asarray(db2, f32)

    def pairs(w):  # [D, F] -> [KJ*128, 2, F]
        return np.ascontiguousarray(
            w.reshape(KJ, 2, 128, F).transpose(0, 2, 1, 3)
            .reshape(KJ * 128, 2, F))

    def pairs4(w):  # [D, F] -> [JJ*128, 4, F]
        return np.ascontiguousarray(
            w.reshape(JJ, 4, 128, F).transpose(0, 2, 1, 3)
            .reshape(JJ * 128, 4, F))

    w1p = pairs4(W1).astype(bf16)
    w1l = np.ascontiguousarray(w1p[:, :, 0:FL])
    w1rr = np.ascontiguousarray(w1p[:, :, FL:F])
    w2b = np.ascontiguousarray(W2).astype(bf16)
    mw1t = np.ascontiguousarray(
        np.asarray(mW1, f32).T.reshape(KF, 128, HID).transpose(1, 0, 2)
        .reshape(128, KF * HID)).astype(bf16)
    mw2tf = np.asarray(mW2, f32).T                       # [HID, T]
    mw2t = np.zeros((128, 2 * T), f32)
    mw2t[:, 0:T] = mw2tf[0:128, :]
    mw2t[0:HID - 128, T:2 * T] = mw2tf[128:HID, :]
    mw2t = mw2t.astype(bf16)
    fsel = np.zeros((33, B), f32)
    for b in range(B):
        fsel[b, b] = 1.0
        fsel[B + b, b] = 1.0 / RS
    fsel[32, :] = 1.0

    in_maps = []
    for c in range(NCORES):
        dwp = (pairs4(dW1[c]) * WS).astype(fp8)
        bc = np.zeros((32, BCW), f32)
        bc[0:B, BC_B1:BC_B1 + F] = b1[None, :]
        bc[0:B, BC_B2:BC_B2 + F] = b2[None, :]
        bc[0:B, BC_DB2:BC_DB2 + F] = db2[c][None, :]
        bc[0:B, BC_MB1:BC_MB1 + HID] = mb1[None, :]
        bc[0:B, BC_MB2:BC_MB2 + T] = mb2[None, :]
        bc[0:B, BC_SEL + c] = 1.0
        if c == 0:
            bc[0:B, BC_FB] = 1.0
        bc[0:32, BC_EYE:BC_EYE + 32] = np.eye(32, dtype=f32)
        in_maps.append({
            "xqr": xqr,
            "xtb": xtb,
            "w1l": w1l,
            "w1r": w1rr,
            "dw1l": np.ascontiguousarray(dwp[:, :, 0:FL]),
            "dw1r": np.ascontiguousarray(dwp[:, :, FL:F]),
            "w2b": w2b,
            "dw2b": np.ascontiguousarray(dW2[c]).astype(bf16),
            "mw1t": mw1t,
            "mw2t": mw2t,
            "fsel": fsel,
            "db1r": db1[c][None, :],
            "bc": bc,
        })
    return in_maps


def run(inputs, trace=False, trace_cores=None, tmpdir=None):
    nc = _get_nc()
    in_maps = _prep_in_maps(**inputs)
    res = bass_utils.run_bass_kernel_spmd(
        nc, in_maps, core_ids=list(range(NCORES)), trace=trace,
        trace_cores=trace_cores, tmpdir=tmpdir)
    acc = res.results[0]["ot"].astype(np.float64)
    for c in range(1, NCORES):
        acc = acc + res.results[c]["ot"].astype(np.float64)
    return acc.astype(np.float32), res


def kernel(**inputs):
    out, _ = run(inputs, trace=False)
    return out
